# revision 31
# baseline (speedup 1.0000x reference)
"""Batched Procrustes-alignment loss on 8 Trainium2 NeuronCores.

Data-parallel over batch (B=262144 -> 32768/core), laid out as [128
partitions, F=256] planes (one scalar per batch element per plane).

Per batch element:
  center pred/target over J=17 joints; per-joint squared norms P2/T2;
  scale s = tn/(pn+eps); H = PC^T TC via streamed per-plane products +
  j-tree reductions; closed-form symmetric 3x3 eigensolver on A = H^T H
  (trigonometric eigenvalues via acos/cos expressed with Arctan+Sin
  activations, eigenvectors via cross-of-rows, v2 = v0 x v1 so det(V)=+1
  and all LAPACK sign bookkeeping cancels); u_i = H v_i / sigma_i,
  u2 = (u0 x u1)/s; G = sum_i u_i (x) m_i scaled by -2s.
  dist_j^2 = s^2 P2_j - 2 s W_j + T2_j with W via rotated-pred planes
  e_r = sum_c Gt_cr PC_c, then d2 += TC_r*e_r; loss = mean sqrt.

Output: per-core per-partition partial sums [128,1]; host sums in float64
and divides by B*J.
"""
import numpy as np
import concourse.bass as bass
import concourse.mybir as mybir
import concourse.tile as tile
from concourse import bacc
from concourse.bass_utils import run_bass_kernel_spmd

AF = mybir.ActivationFunctionType
OP = mybir.AluOpType
AX = mybir.AxisListType
f32 = mybir.dt.float32
bf16 = mybir.dt.bfloat16

B, J, C = 262144, 17, 3
JC = J * C
NCORES = 8
BC = B // NCORES
P = 128
F = 256
JF = J * F
SUB = 64
NSUB = F // SUB
EPS = 1e-8
TINY = 1e-20

# engine assignment knobs ("v" = DVE vector, "g" = gpsimd Pool, "s" = scalar/Act)
KNOBS = dict(
    center=["v", "g", "g", "g", "v", "g"],   # per (tensor*3 + c)
    omult=["g", "v", "v", "g", "v", "v", "g", "v", "v"],
    emult=["v"] * 9,
    eadd=["v"] * 6,
    tcmul=["v"] * 3,
    d2add=["v"] * 3,
    sqadd=["g", "g", "g", "g"],
    uassm="v",
    gassm="v",
    htree="v",
    sq="s",
)


def _ap(t, off, dims):
    a = t[:]
    return bass.AP(a.tensor, a.offset + off, [a.ap[0]] + dims)


def _pl(t, off, n):
    return _ap(t, off, [[1, n]])


def build_nc(iters=1, knobs=None, stop=99):
    kn = dict(KNOBS)
    if knobs:
        kn.update(knobs)

    nc = bacc.Bacc("TRN2", target_bir_lowering=False)
    nc._dbg = {}
    pred_d = nc.dram_tensor("pred", [BC, JC], f32, kind="ExternalInput")
    targ_d = nc.dram_tensor("target", [BC, JC], f32, kind="ExternalInput")
    out_d = nc.dram_tensor("partial", [P, 1], f32, kind="ExternalOutput")

    def E(key):
        v = kn[key] if isinstance(kn[key], str) else None
        assert v is not None
        return {"v": nc.vector, "g": nc.gpsimd, "s": nc.scalar}[v]

    def Ei(key, i):
        return {"v": nc.vector, "g": nc.gpsimd, "s": nc.scalar}[kn[key][i]]

    with tile.TileContext(nc) as tc:
        with (
            tc.tile_pool(name="persist", bufs=1) as persist,
            tc.tile_pool(name="rawp", bufs=1) as rawp,
            tc.tile_pool(name="meanp", bufs=1) as meanp,
            tc.tile_pool(name="pctc", bufs=1) as pctcp,
            tc.tile_pool(name="oring", bufs=1) as oring,
            tc.tile_pool(name="sqp", bufs=1) as sqp,
            tc.tile_pool(name="hp", bufs=1) as hp,
            tc.tile_pool(name="ep", bufs=1) as epool,
            tc.tile_pool(name="late", bufs=1) as late,
            tc.tile_pool(name="thinE", bufs=1) as thinE,
            tc.tile_pool(name="psth", bufs=1, space="PSUM") as psth,
        ):
            acc = persist.tile([P, F], f32, tag="acc", name="acc")
            b2p3 = persist.tile([P, 1], f32, tag="b2p3", name="b2p3")
            b4p3 = persist.tile([P, 1], f32, tag="b4p3", name="b4p3")
            nc.gpsimd.memset(acc[:], 0.0)
            nc.gpsimd.memset(b2p3[:], 2.0943951023931953)
            nc.gpsimd.memset(b4p3[:], 1.0471975511965976)  # pi/3

            def thinE_t():
                return thinE.tile([P, F], f32, tag="te", name="te", bufs=8)

            _ps = {"n": 0, "banks": []}

            def psum_t(tg):
                i = _ps["n"]
                _ps["n"] += 1
                assert i < 16
                if i % 2 == 0:
                    _ps["banks"].append(
                        psth.tile([P, 2 * F], f32, tag=f"pb{i // 2}",
                                  name=f"pb{i // 2}"))
                blk = _ps["banks"][i // 2]
                off = (i % 2) * F

                class _T:
                    def __getitem__(self, _):
                        return _pl(blk, off, F)
                return _T()

            def body():
                _ps["n"] = 0
                _ps["banks"] = []
                # --------- tiles (allocated per iteration; tags reuse slots)
                PC = pctcp.tile([P, 3 * JF], bf16, tag="PC", name="PC")
                TC = pctcp.tile([P, 3 * JF], bf16, tag="TC", name="TC")
                P2 = sqp.tile([P, JF], bf16, tag="P2", name="P2")
                T2 = sqp.tile([P, JF], bf16, tag="T2", name="T2")
                H = hp.tile([P, 9 * F], f32, tag="H", name="H")
                d2 = late.tile([P, JF], bf16, tag="d2", name="d2")
                G = late.tile([P, 9 * F], bf16, tag="G", name="G")
                mtmp = meanp.tile([P, 2304], f32, tag="mt", name="mtmp")
                mean_p = meanp.tile([P, 768], f32, tag="mp", name="mean_p")
                mean_t = meanp.tile([P, 768], f32, tag="mq", name="mean_t")

                # --------- per-sub-block load + mean tree + center
                for ti, (dram, mean, ctr) in enumerate(
                        ((pred_d, mean_p, PC), (targ_d, mean_t, TC))):
                    for s in range(NSUB):
                        raw = rawp.tile([P, JC * SUB], f32, tag="raw",
                                        name="raw", bufs=2)
                        off = (s * SUB) * JC
                        nc.sync.dma_start(
                            raw[:], bass.AP(dram[:].tensor, off,
                                            [[F * JC, P], [1, JC * SUB]]))
                        # mean tree over j (all 3 c at once); u = j*3+c
                        r1, r2, r3, r4 = 0, 1536, 0, 512
                        nc.vector.tensor_tensor(
                            _ap(mtmp, r1, [[24, SUB], [1, 24]]),
                            _ap(raw, 0, [[JC, SUB], [1, 24]]),
                            _ap(raw, 24, [[JC, SUB], [1, 24]]), OP.add)
                        nc.vector.tensor_tensor(
                            _ap(mtmp, r2, [[12, SUB], [1, 12]]),
                            _ap(mtmp, r1, [[24, SUB], [1, 12]]),
                            _ap(mtmp, r1 + 12, [[24, SUB], [1, 12]]), OP.add)
                        nc.vector.tensor_tensor(
                            _ap(mtmp, r3, [[6, SUB], [1, 6]]),
                            _ap(mtmp, r2, [[12, SUB], [1, 6]]),
                            _ap(mtmp, r2 + 6, [[12, SUB], [1, 6]]), OP.add)
                        nc.vector.tensor_tensor(
                            _ap(mtmp, r4, [[3, SUB], [1, 3]]),
                            _ap(mtmp, r3, [[6, SUB], [1, 3]]),
                            _ap(mtmp, r3 + 3, [[6, SUB], [1, 3]]), OP.add)
                        nc.vector.tensor_tensor(
                            _ap(mean, s * SUB * 3, [[3, SUB], [1, 3]]),
                            _ap(mtmp, r4, [[3, SUB], [1, 3]]),
                            _ap(raw, 48, [[JC, SUB], [1, 3]]), OP.add)
                        nc.vector.tensor_scalar_mul(
                            _ap(mean, s * SUB * 3, [[3, SUB], [1, 3]]),
                            _ap(mean, s * SUB * 3, [[3, SUB], [1, 3]]), 1.0 / J)
                        # center: PC_c[j, f] = raw - mean, per c
                        for c in range(3):
                            Ei("center", ti * 3 + c).tensor_tensor(
                                _ap(ctr, c * JF + s * SUB, [[F, J], [1, SUB]]),
                                _ap(raw, c, [[3, J], [JC, SUB]]),
                                _ap(mean, s * SUB * 3 + c, [[0, J], [3, SUB]]),
                                OP.subtract)

                # long-lived thin planes carved out of the e-phase slots:
                # they die at G-assembly, exactly when the e tiles are born,
                # so the raw DMA ring stays decoupled from the SVD tail and
                # iteration k+1's loads overlap iteration k's distance phase.
                tbs_named = [
                    epool.tile([P, 2048], f32, tag="e", name=f"tbn{i}", bufs=3)
                    for i in range(3)
                ]
                nb = {"n": 0}

                def named(tg):
                    i = nb["n"]
                    nb["n"] += 1
                    assert i < 24
                    blk = tbs_named[i // 8]
                    off = (i % 8) * F

                    class _T:
                        def __getitem__(self, _):
                            return _pl(blk, off, F)
                    return _T()

                def cblk(t, c):
                    return _pl(t, c * JF, JF)

                if stop <= 0:
                    return

                # --------- squares -> P2/T2 (Act) + adds (DVE)
                sq1 = sqp.tile([P, JF], bf16, tag="sq", name="sq1", bufs=2)
                nc.scalar.activation(P2[:], cblk(PC, 0), AF.Square)
                nc.scalar.activation(sq1[:], cblk(PC, 1), AF.Square)
                Ei("sqadd", 0).tensor_tensor(P2[:], P2[:], sq1[:], OP.add)
                sq2 = sqp.tile([P, JF], bf16, tag="sq", name="sq2", bufs=2)
                nc.scalar.activation(sq2[:], cblk(PC, 2), AF.Square)
                Ei("sqadd", 1).tensor_tensor(P2[:], P2[:], sq2[:], OP.add)
                nc.scalar.activation(T2[:], cblk(TC, 0), AF.Square)
                sq3 = sqp.tile([P, JF], bf16, tag="sq", name="sq3", bufs=2)
                nc.scalar.activation(sq3[:], cblk(TC, 1), AF.Square)
                Ei("sqadd", 2).tensor_tensor(T2[:], T2[:], sq3[:], OP.add)
                sq4 = sqp.tile([P, JF], bf16, tag="sq", name="sq4", bufs=2)
                nc.scalar.activation(sq4[:], cblk(TC, 2), AF.Square)
                Ei("sqadd", 3).tensor_tensor(T2[:], T2[:], sq4[:], OP.add)
                # sqrt planes for norms
                sp2 = sqp.tile([P, JF], bf16, tag="sq", name="sp2", bufs=2)
                nc.scalar.activation(sp2[:], P2[:], AF.Sqrt)
                st2 = sqp.tile([P, JF], bf16, tag="sq", name="st2", bufs=2)
                nc.scalar.activation(st2[:], T2[:], AF.Sqrt)

                # --------- O products (streamed) + H j-trees
                # H plane (c*3+r) = sum_j PC_c[j] * TC_r[j]
                for cc in range(3):
                    for r in range(3):
                        h = cc * 3 + r
                        O = oring.tile([P, JF], bf16, tag="O", name="O", bufs=1)
                        Ei("omult", h).tensor_tensor(
                            O[:], cblk(PC, cc), cblk(TC, r), OP.mult)
                        ht = hp.tile([P, 8 * F], f32, tag="ht", name="ht", bufs=1)
                        eng = E("htree")
                        eng.tensor_tensor(
                            ht[:], _ap(O, 0, [[F, 8], [1, F]]),
                            _ap(O, 8 * F, [[F, 8], [1, F]]), OP.add)
                        eng.tensor_tensor(
                            _pl(ht, 0, 4 * F), _pl(ht, 0, 4 * F),
                            _pl(ht, 4 * F, 4 * F), OP.add)
                        eng.tensor_tensor(
                            _pl(ht, 0, 2 * F), _pl(ht, 0, 2 * F),
                            _pl(ht, 2 * F, 2 * F), OP.add)
                        eng.tensor_tensor(
                            _pl(ht, 0, F), _pl(ht, 0, F), _pl(ht, F, F), OP.add)
                        eng.tensor_tensor(
                            _pl(H, h * F, F), _pl(ht, 0, F),
                            _pl(O, 16 * F, F), OP.add)

                def Hp(r, cc):
                    return _pl(H, (cc * 3 + r) * F, F)

                # --------- norm trees (pn from sp2, tn from st2)
                def ntree(srcpl, out):
                    ht = hp.tile([P, 8 * F], f32, tag="ht", name="nt", bufs=1)
                    nc.vector.tensor_tensor(
                        ht[:], _ap(srcpl, 0, [[F, 8], [1, F]]),
                        _ap(srcpl, 8 * F, [[F, 8], [1, F]]), OP.add)
                    nc.vector.tensor_tensor(
                        _pl(ht, 0, 4 * F), _pl(ht, 0, 4 * F),
                        _pl(ht, 4 * F, 4 * F), OP.add)
                    nc.vector.tensor_tensor(
                        _pl(ht, 0, 2 * F), _pl(ht, 0, 2 * F),
                        _pl(ht, 2 * F, 2 * F), OP.add)
                    nc.vector.tensor_tensor(
                        _pl(ht, 0, F), _pl(ht, 0, F), _pl(ht, F, F), OP.add)
                    nc.vector.tensor_tensor(
                        out[:], _pl(ht, 0, F), _pl(srcpl, 16 * F, F), OP.add)

                if stop <= 1:
                    return
                # --------- A = H^T H (6 upper entries), thin
                A6 = {}
                for (a, b) in ((0, 0), (0, 1), (0, 2), (1, 1), (1, 2), (2, 2)):
                    t1 = thinE_t()
                    nc.vector.tensor_tensor(t1[:], Hp(a, 0), Hp(b, 0), OP.mult)
                    t2 = thinE_t()
                    nc.vector.tensor_tensor(t2[:], Hp(a, 1), Hp(b, 1), OP.mult)
                    nc.vector.tensor_tensor(t1[:], t1[:], t2[:], OP.add)
                    t3 = thinE_t()
                    nc.vector.tensor_tensor(t3[:], Hp(a, 2), Hp(b, 2), OP.mult)
                    At = named(f"A{a}{b}")
                    nc.vector.tensor_tensor(At[:], t1[:], t3[:], OP.add)
                    A6[(a, b)] = At
                a00, a01, a02 = A6[(0, 0)], A6[(0, 1)], A6[(0, 2)]
                a11, a12, a22 = A6[(1, 1)], A6[(1, 2)], A6[(2, 2)]

                # --------- eigenvalues (closed form)
                q3 = thinE_t()
                nc.vector.tensor_tensor(q3[:], a00[:], a11[:], OP.add)
                nc.vector.tensor_tensor(q3[:], q3[:], a22[:], OP.add)
                m01, g0, g1 = named("m01"), named("g0"), named("g1")
                g2 = named("g2")
                nc.vector.tensor_tensor(m01[:], a01[:], a01[:], OP.mult)
                nc.vector.tensor_tensor(g0[:], a01[:], a12[:], OP.mult)
                nc.vector.tensor_tensor(g1[:], a01[:], a02[:], OP.mult)
                nc.vector.tensor_tensor(g2[:], a02[:], a12[:], OP.mult)
                m02 = thinE_t()
                nc.vector.tensor_tensor(m02[:], a02[:], a02[:], OP.mult)
                m12 = thinE_t()
                nc.vector.tensor_tensor(m12[:], a12[:], a12[:], OP.mult)
                p1 = thinE_t()
                nc.vector.tensor_tensor(p1[:], m01[:], m02[:], OP.add)
                nc.vector.tensor_tensor(p1[:], p1[:], m12[:], OP.add)
                q = named("q")
                nc.vector.tensor_scalar_mul(q[:], q3[:], 1.0 / 3)
                b00, b11, b22 = thinE_t(), thinE_t(), thinE_t()
                nc.vector.tensor_tensor(b00[:], a00[:], q[:], OP.subtract)
                nc.vector.tensor_tensor(b11[:], a11[:], q[:], OP.subtract)
                nc.vector.tensor_tensor(b22[:], a22[:], q[:], OP.subtract)
                p2s = thinE_t()
                nc.vector.tensor_tensor(p2s[:], b00[:], b00[:], OP.mult)
                tb = thinE_t()
                nc.vector.tensor_tensor(tb[:], b11[:], b11[:], OP.mult)
                nc.vector.tensor_tensor(p2s[:], p2s[:], tb[:], OP.add)
                nc.vector.tensor_tensor(tb[:], b22[:], b22[:], OP.mult)
                nc.vector.tensor_tensor(p2s[:], p2s[:], tb[:], OP.add)
                nc.vector.scalar_tensor_tensor(
                    p2s[:], p1[:], 2.0, p2s[:], OP.mult, OP.add)
                pA = named("pA")
                nc.scalar.activation(pA[:], p2s[:], AF.Sqrt, scale=1.0 / 6)
                # fill: detB terms (independent of pA)
                c0 = thinE_t()
                nc.vector.tensor_tensor(c0[:], b11[:], b22[:], OP.mult)
                nc.vector.tensor_tensor(c0[:], c0[:], m12[:], OP.subtract)
                c1 = thinE_t()
                nc.vector.tensor_tensor(c1[:], a01[:], b22[:], OP.mult)
                nc.vector.tensor_tensor(c1[:], c1[:], g2[:], OP.subtract)
                c2 = thinE_t()
                nc.vector.tensor_tensor(c2[:], b11[:], a02[:], OP.mult)
                nc.vector.tensor_tensor(c2[:], g0[:], c2[:], OP.subtract)
                detB = thinE_t()
                nc.vector.tensor_tensor(detB[:], b00[:], c0[:], OP.mult)
                tdb = thinE_t()
                nc.vector.tensor_tensor(tdb[:], a01[:], c1[:], OP.mult)
                nc.vector.tensor_tensor(detB[:], detB[:], tdb[:], OP.subtract)
                nc.vector.tensor_tensor(tdb[:], a02[:], c2[:], OP.mult)
                nc.vector.tensor_tensor(detB[:], detB[:], tdb[:], OP.add)
                pinv = thinE_t()
                nc.vector.tensor_scalar_add(pinv[:], pA[:], TINY)
                nc.vector.reciprocal_approx_fast(pinv[:], pinv[:])
                p3 = thinE_t()
                nc.vector.tensor_tensor(p3[:], pinv[:], pinv[:], OP.mult)
                nc.vector.tensor_tensor(p3[:], p3[:], pinv[:], OP.mult)
                rc = thinE_t()
                nc.vector.tensor_tensor(rc[:], detB[:], p3[:], OP.mult)
                nc.vector.tensor_scalar(rc[:], rc[:], 0.5, 1.0, OP.mult, OP.min)
                nc.vector.tensor_scalar_max(rc[:], rc[:], -1.0)
                rr = thinE_t()
                nc.vector.tensor_tensor(rr[:], rc[:], rc[:], OP.mult)
                wA = thinE_t()
                nc.scalar.activation(wA[:], rr[:], AF.Sqrt, bias=1.0, scale=-1.0)
                # fill: pn tree
                pn = psum_t("pn")
                ntree(sp2, pn)
                rat = thinE_t()
                nc.vector.tensor_scalar_add(rat[:], wA[:], 1e-10)
                nc.vector.reciprocal_approx_fast(rat[:], rat[:])
                nc.vector.tensor_tensor(rat[:], rc[:], rat[:], OP.mult)
                # atan with range reduction: |x|>1 -> sign(x)*pi/2 - atan(1/x)
                a1 = thinE_t()
                nc.vector.tensor_scalar(a1[:], rat[:], 1.0, -1.0, OP.min, OP.max)
                rat2 = thinE_t()
                nc.vector.tensor_tensor(rat2[:], rat[:], rat[:], OP.mult)
                rinv = thinE_t()
                nc.vector.tensor_scalar_add(rinv[:], rat2[:], TINY)
                nc.vector.reciprocal_approx_fast(rinv[:], rinv[:])
                nc.vector.tensor_tensor(rinv[:], rat[:], rinv[:], OP.mult)
                nc.vector.tensor_scalar(rinv[:], rinv[:], 1.0, -1.0, OP.min, OP.max)
                sg = thinE_t()
                nc.vector.tensor_scalar(sg[:], rat[:], 1e10, 1.0, OP.mult, OP.min)
                nc.vector.tensor_scalar_max(sg[:], sg[:], -1.0)
                at1 = thinE_t()
                nc.scalar.activation(at1[:], a1[:], AF.Arctan)
                at2 = thinE_t()
                nc.scalar.activation(at2[:], rinv[:], AF.Arctan)
                atb = thinE_t()
                nc.vector.scalar_tensor_tensor(
                    atb[:], sg[:], 1.5707963267948966, at2[:],
                    OP.mult, OP.subtract)
                m_ = thinE_t()
                nc.vector.tensor_scalar_add(m_[:], rat2[:], -1.0)
                nc.vector.tensor_scalar(m_[:], m_[:], 1e10, 1.0, OP.mult, OP.min)
                nc.vector.tensor_scalar_max(m_[:], m_[:], 0.0)
                atn = thinE_t()
                nc.vector.tensor_tensor(atn[:], atb[:], at1[:], OP.subtract)
                nc.vector.tensor_tensor(atn[:], atn[:], m_[:], OP.mult)
                nc.vector.tensor_tensor(atn[:], atn[:], at1[:], OP.add)
                # fill: tn tree
                tn = psum_t("tn")
                ntree(st2, tn)
                cs1 = psum_t("cs1")
                nc.scalar.activation(cs1[:], atn[:], AF.Sin,
                                     bias=b2p3[:], scale=-1.0 / 3)
                cs2 = psum_t("cs2")
                nc.scalar.activation(cs2[:], atn[:], AF.Sin,
                                     bias=b4p3[:], scale=-1.0 / 3)
                # fill: s, s2, P~2 = s^2*P2 into d2 (f32), then d2 += T2
                sS = named("sS")
                nc.vector.tensor_scalar_add(sS[:], pn[:], EPS)
                nc.vector.reciprocal_approx_fast(sS[:], sS[:])
                nc.vector.tensor_tensor(sS[:], sS[:], tn[:], OP.mult)
                s2 = psum_t("s2")
                nc.vector.tensor_tensor(s2[:], sS[:], sS[:], OP.mult)
                nc.vector.tensor_tensor(
                    d2[:], P2[:], _ap(s2, 0, [[0, J], [1, F]]), OP.mult)
                nc.vector.tensor_tensor(d2[:], d2[:], T2[:], OP.add)
                lam0, lam1 = psum_t("lam0"), psum_t("lam1")
                tp = thinE_t()
                nc.vector.tensor_tensor(tp[:], pA[:], cs1[:], OP.mult)
                nc.vector.scalar_tensor_tensor(
                    lam0[:], tp[:], 2.0, q[:], OP.mult, OP.add)
                lam2 = thinE_t()
                nc.vector.tensor_tensor(tp[:], pA[:], cs2[:], OP.mult)
                nc.vector.scalar_tensor_tensor(
                    lam2[:], tp[:], -2.0, q[:], OP.mult, OP.add)
                nc.vector.scalar_tensor_tensor(
                    lam1[:], q[:], 3.0, lam0[:], OP.mult, OP.subtract)
                nc.vector.tensor_tensor(lam1[:], lam1[:], lam2[:], OP.subtract)

                # --------- eigenvectors v0 (lam0), v1 (lam1); v2 = v0 x v1
                def eigvec(lam, pref):
                    vx = named(pref + "x")
                    vy = named(pref + "y")
                    vz = named(pref + "z")
                    b0 = thinE_t()
                    nc.vector.tensor_tensor(b0[:], a00[:], lam[:], OP.subtract)
                    b1 = thinE_t()
                    nc.vector.tensor_tensor(b1[:], a11[:], lam[:], OP.subtract)
                    nc.vector.tensor_tensor(vx[:], a02[:], b1[:], OP.mult)
                    nc.vector.tensor_tensor(vx[:], g0[:], vx[:], OP.subtract)
                    nc.vector.tensor_tensor(vy[:], b0[:], a12[:], OP.mult)
                    nc.vector.tensor_tensor(vy[:], g1[:], vy[:], OP.subtract)
                    nc.vector.tensor_tensor(vz[:], b0[:], b1[:], OP.mult)
                    nc.vector.tensor_tensor(vz[:], vz[:], m01[:], OP.subtract)
                    n2 = thinE_t()
                    nc.vector.tensor_tensor(n2[:], vx[:], vx[:], OP.mult)
                    t2_ = thinE_t()
                    nc.vector.tensor_tensor(t2_[:], vy[:], vy[:], OP.mult)
                    nc.vector.tensor_tensor(n2[:], n2[:], t2_[:], OP.add)
                    nc.vector.tensor_tensor(t2_[:], vz[:], vz[:], OP.mult)
                    nc.vector.tensor_tensor(n2[:], n2[:], t2_[:], OP.add)
                    ns = thinE_t()
                    nc.scalar.activation(ns[:], n2[:], AF.Sqrt)
                    nc.vector.tensor_scalar_add(ns[:], ns[:], TINY)
                    nc.vector.reciprocal_approx_fast(ns[:], ns[:])
                    nc.vector.tensor_tensor(vx[:], vx[:], ns[:], OP.mult)
                    nc.vector.tensor_tensor(vy[:], vy[:], ns[:], OP.mult)
                    nc.vector.tensor_tensor(vz[:], vz[:], ns[:], OP.mult)
                    return vx, vy, vz

                v0 = eigvec(lam0, "v0")
                v1 = eigvec(lam1, "v1")
                v2 = (named("v2x"), named("v2y"), named("v2z"))
                cr = ((1, 2), (2, 0), (0, 1))
                for r in range(3):
                    i1, i2 = cr[r]
                    t1 = thinE_t()
                    nc.vector.tensor_tensor(t1[:], v0[i1][:], v1[i2][:], OP.mult)
                    t2_ = thinE_t()
                    nc.vector.tensor_tensor(t2_[:], v0[i2][:], v1[i1][:], OP.mult)
                    nc.vector.tensor_tensor(v2[r][:], t1[:], t2_[:], OP.subtract)

                # --------- rsig_i = s / sigma_i ; u_i = H v_i * rsig_i
                rsig = []
                for i, lam in enumerate((lam0, lam1)):
                    rl = thinE_t()
                    nc.scalar.activation(rl[:], lam[:], AF.Relu)
                    sg = thinE_t()
                    nc.scalar.activation(sg[:], rl[:], AF.Sqrt)
                    nc.vector.tensor_scalar_add(sg[:], sg[:], TINY)
                    nc.vector.reciprocal_approx_fast(sg[:], sg[:])
                    rs = psum_t(f"rs{i}")
                    nc.vector.scalar_tensor_tensor(
                        rs[:], sg[:], -2.0, sS[:], OP.mult, OP.mult)
                    rsig.append(rs)

                ub = meanp.tile([P, 2304], f32, tag="mt", name="ublock")
                u0 = _ap(ub, 0, [[F, 3], [1, F]])
                u1 = _ap(ub, 3 * F, [[F, 3], [1, F]])
                u2 = _ap(ub, 6 * F, [[F, 3], [1, F]])

                def up(ui, r):
                    return _pl(ub, ui * 3 * F + r * F, F)

                def bc3(t):
                    return _ap(t, 0, [[0, 3], [1, F]])

                def HCg(k):
                    # H rows group for fixed k: planes (c*3+k)... careful:
                    # u_i[r] = sum_k H[r,k] v_i[k]; H[r,k] stored plane (r*3+k)?
                    # H plane (c*3+r) = H_cr = sum_j PC_c TC_r -> H[c,r].
                    # reference H_ik = sum_j pc_{j,i} tc_{j,k} -> H[i,k] = plane(i*3+k)
                    # u_i[r] = sum_k H[r,k] (v_i)_k: planes (r*3+k), r varies
                    # group for fixed k over r: offset k*F stride 3F
                    return _ap(H, k * F, [[3 * F, 3], [1, F]])

                uga = {"v": nc.vector, "g": nc.gpsimd}[kn["uassm"]]
                for i, (vv, rs) in enumerate(((v0, rsig[0]), (v1, rsig[1]))):
                    udst = (u0, u1)[i]
                    gt = meanp.tile([P, 768], f32, tag="mp", name="gt", bufs=1)
                    uga.tensor_tensor(udst, HCg(0), bc3(vv[0]), OP.mult)
                    uga.tensor_tensor(gt[:], HCg(1), bc3(vv[1]), OP.mult)
                    uga.tensor_tensor(udst, udst, gt[:], OP.add)
                    uga.tensor_tensor(gt[:], HCg(2), bc3(vv[2]), OP.mult)
                    uga.tensor_tensor(udst, udst, gt[:], OP.add)
                    uga.tensor_tensor(udst, udst, bc3(rs), OP.mult)
                # u2 = cross(u0, u1) / s
                invs = psum_t("invs")
                nc.vector.tensor_scalar_add(invs[:], sS[:], TINY)
                nc.vector.reciprocal_approx_fast(invs[:], invs[:])
                nc.vector.tensor_scalar_mul(invs[:], invs[:], -0.5)
                for r in range(3):
                    i1, i2 = cr[r]
                    t1 = thinE_t()
                    nc.vector.tensor_tensor(t1[:], up(0, i1), up(1, i2), OP.mult)
                    t2_ = thinE_t()
                    nc.vector.tensor_tensor(t2_[:], up(0, i2), up(1, i1), OP.mult)
                    nc.vector.tensor_tensor(t1[:], t1[:], t2_[:], OP.subtract)
                    nc.vector.tensor_tensor(up(2, r), t1[:], invs[:], OP.mult)

                # --------- G: plane (c*3+r) = sum_i u_i[r] * (v_c)_i, then *-2
                gga = {"v": nc.vector, "g": nc.gpsimd}[kn["gassm"]]
                vs = (v0, v1, v2)
                greps = {}
                for cc in range(3):
                    Gc = _ap(G, cc * 3 * F, [[F, 3], [1, F]])
                    gt = meanp.tile([P, 768], f32, tag="mp", name="gt2", bufs=1)
                    gt2 = meanp.tile([P, 768], f32, tag="mq", name="gt3", bufs=1)
                    gga.tensor_tensor(gt[:], u0, bc3(vs[cc][0]), OP.mult)
                    gga.tensor_tensor(gt2[:], u1, bc3(vs[cc][1]), OP.mult)
                    gga.tensor_tensor(gt[:], gt[:], gt2[:], OP.add)
                    gga.tensor_tensor(gt2[:], u2, bc3(vs[cc][2]), OP.mult)
                    gga.tensor_tensor(Gc, gt[:], gt2[:], OP.add)
                    # replicate this block's planes (r=cc, c=0..2) over j via DMA
                    for c_ in range(3):
                        gr = (oring.tile([P, JF], bf16, tag="O", name="gr",
                                         bufs=1) if c_ == 0 else
                              sqp.tile([P, JF], bf16, tag="sq", name="gr",
                                       bufs=2))
                        nc.sync.dma_start(
                            gr[:], _ap(G, (cc * 3 + c_) * F, [[0, J], [1, F]]))
                        greps[(c_, cc)] = gr

                if stop <= 2:
                    return
                # --------- e_r = sum_c Gt[c*3+r] (bcast over j) * PC_c
                def Gb(cc, r):
                    # G plane (a*3+b) holds (U M)_{b,a}; e_r needs (U M)_{cc,r}
                    if kn.get("edummy"):
                        return cblk(TC, cc)  # timing-only: plain operand
                    return _ap(G, (r * 3 + cc) * F, [[0, J], [1, F]])

                # plain mults from DMA-replicated G planes
                for r in range(3):
                    er = epool.tile([P, JF], bf16, tag="e", name="er", bufs=3)
                    tmp = epool.tile([P, JF], bf16, tag="e", name="etmp", bufs=3)
                    nc.vector.tensor_tensor(
                        er[:], cblk(PC, 0), greps[(0, r)][:], OP.mult)
                    nc.vector.tensor_tensor(
                        tmp[:], cblk(PC, 1), greps[(1, r)][:], OP.mult)
                    nc.vector.tensor_tensor(er[:], er[:], tmp[:], OP.add)
                    nc.vector.tensor_tensor(
                        tmp[:], cblk(PC, 2), greps[(2, r)][:], OP.mult)
                    nc.vector.tensor_tensor(er[:], er[:], tmp[:], OP.add)
                    Ei("tcmul", r).tensor_tensor(
                        er[:], er[:], cblk(TC, r), OP.mult)
                    Ei("d2add", r).tensor_tensor(d2[:], d2[:], er[:], OP.add)

                if stop <= 3:
                    return
                # --------- dist = sqrt(relu(d2)); sum over j; accumulate
                dr = sqp.tile([P, JF], bf16, tag="sq", name="dr", bufs=2)
                nc.scalar.activation(dr[:], d2[:], AF.Relu)
                nc.scalar.activation(dr[:], dr[:], AF.Sqrt)
                dsum = thinE_t()
                ntree(dr, dsum)
                nc.vector.tensor_tensor(acc[:], acc[:], dsum[:], OP.add)

            if iters == 1:
                body()
            else:
                with tc.For_i(0, iters, 1):
                    body()

            accs = persist.tile([P, 1], f32, tag="accs", name="accs")
            nc.vector.tensor_reduce(accs[:], acc[:], axis=AX.X, op=OP.add)
            nc.sync.dma_start(out_d[:], accs[:])

    nc.compile()
    return nc


_nc_cache = None


def get_nc():
    global _nc_cache
    if _nc_cache is None:
        _nc_cache = build_nc()
    return _nc_cache


def run(nc, pred, target, trace=False, **kw):
    pred2 = np.ascontiguousarray(np.asarray(pred), np.float32).reshape(B, JC)
    targ2 = np.ascontiguousarray(np.asarray(target), np.float32).reshape(B, JC)
    in_maps = [
        {"pred": pred2[c * BC:(c + 1) * BC], "target": targ2[c * BC:(c + 1) * BC]}
        for c in range(NCORES)
    ]
    res = run_bass_kernel_spmd(nc, in_maps, list(range(NCORES)), trace=trace, **kw)
    total = sum(r["partial"].astype(np.float64).sum() for r in res.results)
    loss = np.float32(total / (B * J))
    return loss, res


def kernel(pred, target):
    loss, _ = run(get_nc(), pred, target)
    return loss


# revision 32
# speedup vs baseline: 1.2906x; 1.2906x over previous
"""Batched Procrustes-alignment loss on 8 Trainium2 NeuronCores.

Data-parallel over batch (B=262144 -> 32768/core), laid out as [128
partitions, F=256] planes (one scalar per batch element per plane).

Per batch element:
  center pred/target over J=17 joints; per-joint squared norms P2/T2;
  scale s = tn/(pn+eps); H = PC^T TC via streamed per-plane products +
  j-tree reductions; closed-form symmetric 3x3 eigensolver on A = H^T H
  (trigonometric eigenvalues via acos/cos expressed with Arctan+Sin
  activations, eigenvectors via cross-of-rows, v2 = v0 x v1 so det(V)=+1
  and all LAPACK sign bookkeeping cancels); u_i = H v_i / sigma_i,
  u2 = (u0 x u1)/s; G = sum_i u_i (x) m_i scaled by -2s.
  dist_j^2 = s^2 P2_j - 2 s W_j + T2_j with W via rotated-pred planes
  e_r = sum_c Gt_cr PC_c, then d2 += TC_r*e_r; loss = mean sqrt.

Output: per-core per-partition partial sums [128,1]; host sums in float64
and divides by B*J.
"""
import numpy as np
import concourse.bass as bass
import concourse.mybir as mybir
import concourse.tile as tile
from concourse import bacc
from concourse.bass_utils import run_bass_kernel_spmd

AF = mybir.ActivationFunctionType
OP = mybir.AluOpType
AX = mybir.AxisListType
f32 = mybir.dt.float32
bf16 = mybir.dt.bfloat16

B, J, C = 262144, 17, 3
JC = J * C
NCORES = 8
BC = B // NCORES
P = 128
F = 256
JF = J * F
SUB = 64
NSUB = F // SUB
EPS = 1e-8
TINY = 1e-20

# engine assignment knobs ("v" = DVE vector, "g" = gpsimd Pool, "s" = scalar/Act)
KNOBS = dict(
    center=["v", "g", "g", "g", "v", "g"],   # per (tensor*3 + c)
    omult=["v"] * 9,
    emult=["v"] * 9,
    eadd=["v"] * 6,
    tcmul=["v"] * 3,
    d2add=["v"] * 3,
    sqadd=["v", "v", "v", "v"],
    uassm="v",
    gassm="v",
    htree="v",
    sq="s",
)


def _ap(t, off, dims):
    a = t[:]
    return bass.AP(a.tensor, a.offset + off, [a.ap[0]] + dims)


def _pl(t, off, n):
    return _ap(t, off, [[1, n]])


def build_nc(iters=1, knobs=None, stop=99):
    kn = dict(KNOBS)
    if knobs:
        kn.update(knobs)

    nc = bacc.Bacc("TRN2", target_bir_lowering=False)
    nc._dbg = {}
    pred_d = nc.dram_tensor("pred", [BC, JC], f32, kind="ExternalInput")
    targ_d = nc.dram_tensor("target", [BC, JC], f32, kind="ExternalInput")
    out_d = nc.dram_tensor("partial", [P, 1], f32, kind="ExternalOutput")

    def E(key):
        v = kn[key] if isinstance(kn[key], str) else None
        assert v is not None
        return {"v": nc.vector, "g": nc.gpsimd, "s": nc.scalar}[v]

    def Ei(key, i):
        return {"v": nc.vector, "g": nc.gpsimd, "s": nc.scalar}[kn[key][i]]

    with tile.TileContext(nc) as tc:
        with (
            tc.tile_pool(name="persist", bufs=1) as persist,
            tc.tile_pool(name="rawp", bufs=1) as rawp,
            tc.tile_pool(name="meanp", bufs=1) as meanp,
            tc.tile_pool(name="pctc", bufs=1) as pctcp,
            tc.tile_pool(name="oring", bufs=1) as oring,
            tc.tile_pool(name="sqp", bufs=1) as sqp,
            tc.tile_pool(name="hp", bufs=1) as hp,
            tc.tile_pool(name="ep", bufs=1) as epool,
            tc.tile_pool(name="late", bufs=1) as late,
            tc.tile_pool(name="thinE", bufs=1) as thinE,
            tc.tile_pool(name="psth", bufs=1, space="PSUM") as psth,
        ):
            acc = persist.tile([P, F], f32, tag="acc", name="acc")
            b2p3 = persist.tile([P, 1], f32, tag="b2p3", name="b2p3")
            b4p3 = persist.tile([P, 1], f32, tag="b4p3", name="b4p3")
            nc.gpsimd.memset(acc[:], 0.0)
            nc.gpsimd.memset(b2p3[:], 2.0943951023931953)
            nc.gpsimd.memset(b4p3[:], 1.0471975511965976)  # pi/3

            def thinE_t():
                return thinE.tile([P, F], f32, tag="te", name="te", bufs=8)

            _ps = {"n": 0, "banks": []}

            def psum_t(tg):
                i = _ps["n"]
                _ps["n"] += 1
                assert i < 16
                if i % 2 == 0:
                    _ps["banks"].append(
                        psth.tile([P, 2 * F], f32, tag=f"pb{i // 2}",
                                  name=f"pb{i // 2}"))
                blk = _ps["banks"][i // 2]
                off = (i % 2) * F

                class _T:
                    def __getitem__(self, _):
                        return _pl(blk, off, F)
                return _T()

            def body():
                _ps["n"] = 0
                _ps["banks"] = []
                # --------- tiles (allocated per iteration; tags reuse slots)
                PC = pctcp.tile([P, 3 * JF], bf16, tag="PC", name="PC")
                TC = pctcp.tile([P, 3 * JF], bf16, tag="TC", name="TC")
                P2 = sqp.tile([P, JF], bf16, tag="P2", name="P2")
                T2 = sqp.tile([P, JF], bf16, tag="T2", name="T2")
                H = hp.tile([P, 9 * F], f32, tag="H", name="H")
                d2 = late.tile([P, JF], bf16, tag="d2", name="d2")
                G = late.tile([P, 9 * F], bf16, tag="G", name="G")
                mtmp = meanp.tile([P, 2304], f32, tag="mt", name="mtmp")
                mean_p = meanp.tile([P, 768], f32, tag="mp", name="mean_p")
                mean_t = meanp.tile([P, 768], f32, tag="mq", name="mean_t")

                # --------- per-sub-block load + mean tree + center
                for ti, (dram, mean, ctr) in enumerate(
                        ((pred_d, mean_p, PC), (targ_d, mean_t, TC))):
                    for s in range(NSUB):
                        raw = rawp.tile([P, JC * SUB], f32, tag="raw",
                                        name="raw", bufs=2)
                        off = (s * SUB) * JC
                        nc.sync.dma_start(
                            raw[:], bass.AP(dram[:].tensor, off,
                                            [[F * JC, P], [1, JC * SUB]]))
                        # mean tree over j (all 3 c at once); u = j*3+c
                        r1, r2, r3, r4 = 0, 1536, 0, 512
                        nc.vector.tensor_tensor(
                            _ap(mtmp, r1, [[24, SUB], [1, 24]]),
                            _ap(raw, 0, [[JC, SUB], [1, 24]]),
                            _ap(raw, 24, [[JC, SUB], [1, 24]]), OP.add)
                        nc.vector.tensor_tensor(
                            _ap(mtmp, r2, [[12, SUB], [1, 12]]),
                            _ap(mtmp, r1, [[24, SUB], [1, 12]]),
                            _ap(mtmp, r1 + 12, [[24, SUB], [1, 12]]), OP.add)
                        nc.vector.tensor_tensor(
                            _ap(mtmp, r3, [[6, SUB], [1, 6]]),
                            _ap(mtmp, r2, [[12, SUB], [1, 6]]),
                            _ap(mtmp, r2 + 6, [[12, SUB], [1, 6]]), OP.add)
                        nc.vector.tensor_tensor(
                            _ap(mtmp, r4, [[3, SUB], [1, 3]]),
                            _ap(mtmp, r3, [[6, SUB], [1, 3]]),
                            _ap(mtmp, r3 + 3, [[6, SUB], [1, 3]]), OP.add)
                        nc.vector.tensor_tensor(
                            _ap(mean, s * SUB * 3, [[3, SUB], [1, 3]]),
                            _ap(mtmp, r4, [[3, SUB], [1, 3]]),
                            _ap(raw, 48, [[JC, SUB], [1, 3]]), OP.add)
                        nc.vector.tensor_scalar_mul(
                            _ap(mean, s * SUB * 3, [[3, SUB], [1, 3]]),
                            _ap(mean, s * SUB * 3, [[3, SUB], [1, 3]]), 1.0 / J)
                        # center: PC_c[j, f] = raw - mean, per c
                        for c in range(3):
                            Ei("center", ti * 3 + c).tensor_tensor(
                                _ap(ctr, c * JF + s * SUB, [[F, J], [1, SUB]]),
                                _ap(raw, c, [[3, J], [JC, SUB]]),
                                _ap(mean, s * SUB * 3 + c, [[0, J], [3, SUB]]),
                                OP.subtract)

                # long-lived thin planes carved out of the e-phase slots:
                # they die at G-assembly, exactly when the e tiles are born,
                # so the raw DMA ring stays decoupled from the SVD tail and
                # iteration k+1's loads overlap iteration k's distance phase.
                tbs_named = [
                    epool.tile([P, 2048], f32, tag="e", name=f"tbn{i}", bufs=3)
                    for i in range(3)
                ]
                nb = {"n": 0}

                def named(tg):
                    i = nb["n"]
                    nb["n"] += 1
                    assert i < 24
                    blk = tbs_named[i // 8]
                    off = (i % 8) * F

                    class _T:
                        def __getitem__(self, _):
                            return _pl(blk, off, F)
                    return _T()

                def cblk(t, c):
                    return _pl(t, c * JF, JF)

                if stop <= 0:
                    return

                # --------- squares -> P2/T2 (Act) + adds (DVE)
                sq1 = sqp.tile([P, JF], bf16, tag="sq", name="sq1", bufs=2)
                nc.scalar.activation(P2[:], cblk(PC, 0), AF.Square)
                nc.scalar.activation(sq1[:], cblk(PC, 1), AF.Square)
                Ei("sqadd", 0).tensor_tensor(P2[:], P2[:], sq1[:], OP.add)
                sq2 = sqp.tile([P, JF], bf16, tag="sq", name="sq2", bufs=2)
                nc.scalar.activation(sq2[:], cblk(PC, 2), AF.Square)
                Ei("sqadd", 1).tensor_tensor(P2[:], P2[:], sq2[:], OP.add)
                nc.scalar.activation(T2[:], cblk(TC, 0), AF.Square)
                sq3 = sqp.tile([P, JF], bf16, tag="sq", name="sq3", bufs=2)
                nc.scalar.activation(sq3[:], cblk(TC, 1), AF.Square)
                Ei("sqadd", 2).tensor_tensor(T2[:], T2[:], sq3[:], OP.add)
                sq4 = sqp.tile([P, JF], bf16, tag="sq", name="sq4", bufs=2)
                nc.scalar.activation(sq4[:], cblk(TC, 2), AF.Square)
                Ei("sqadd", 3).tensor_tensor(T2[:], T2[:], sq4[:], OP.add)
                # sqrt planes for norms
                sp2 = sqp.tile([P, JF], bf16, tag="sq", name="sp2", bufs=2)
                nc.scalar.activation(sp2[:], P2[:], AF.Sqrt)
                st2 = sqp.tile([P, JF], bf16, tag="sq", name="st2", bufs=2)
                nc.scalar.activation(st2[:], T2[:], AF.Sqrt)

                # --------- O products (streamed) + H j-trees
                # H plane (c*3+r) = sum_j PC_c[j] * TC_r[j]
                for cc in range(3):
                    for r in range(3):
                        h = cc * 3 + r
                        O = oring.tile([P, JF], bf16, tag="O", name="O", bufs=1)
                        Ei("omult", h).tensor_tensor(
                            O[:], cblk(PC, cc), cblk(TC, r), OP.mult)
                        ht = hp.tile([P, 8 * F], f32, tag="ht", name="ht", bufs=1)
                        eng = E("htree")
                        eng.tensor_tensor(
                            ht[:], _ap(O, 0, [[F, 8], [1, F]]),
                            _ap(O, 8 * F, [[F, 8], [1, F]]), OP.add)
                        eng.tensor_tensor(
                            _pl(ht, 0, 4 * F), _pl(ht, 0, 4 * F),
                            _pl(ht, 4 * F, 4 * F), OP.add)
                        eng.tensor_tensor(
                            _pl(ht, 0, 2 * F), _pl(ht, 0, 2 * F),
                            _pl(ht, 2 * F, 2 * F), OP.add)
                        eng.tensor_tensor(
                            _pl(ht, 0, F), _pl(ht, 0, F), _pl(ht, F, F), OP.add)
                        eng.tensor_tensor(
                            _pl(H, h * F, F), _pl(ht, 0, F),
                            _pl(O, 16 * F, F), OP.add)

                def Hp(r, cc):
                    return _pl(H, (cc * 3 + r) * F, F)

                # --------- norm trees (pn from sp2, tn from st2)
                def ntree(srcpl, out):
                    ht = hp.tile([P, 8 * F], f32, tag="ht", name="nt", bufs=1)
                    nc.vector.tensor_tensor(
                        ht[:], _ap(srcpl, 0, [[F, 8], [1, F]]),
                        _ap(srcpl, 8 * F, [[F, 8], [1, F]]), OP.add)
                    nc.vector.tensor_tensor(
                        _pl(ht, 0, 4 * F), _pl(ht, 0, 4 * F),
                        _pl(ht, 4 * F, 4 * F), OP.add)
                    nc.vector.tensor_tensor(
                        _pl(ht, 0, 2 * F), _pl(ht, 0, 2 * F),
                        _pl(ht, 2 * F, 2 * F), OP.add)
                    nc.vector.tensor_tensor(
                        _pl(ht, 0, F), _pl(ht, 0, F), _pl(ht, F, F), OP.add)
                    nc.vector.tensor_tensor(
                        out[:], _pl(ht, 0, F), _pl(srcpl, 16 * F, F), OP.add)

                if stop <= 1:
                    return
                # --------- A = H^T H (6 upper entries), thin
                A6 = {}
                for (a, b) in ((0, 0), (0, 1), (0, 2), (1, 1), (1, 2), (2, 2)):
                    t1 = thinE_t()
                    nc.vector.tensor_tensor(t1[:], Hp(a, 0), Hp(b, 0), OP.mult)
                    t2 = thinE_t()
                    nc.vector.tensor_tensor(t2[:], Hp(a, 1), Hp(b, 1), OP.mult)
                    nc.vector.tensor_tensor(t1[:], t1[:], t2[:], OP.add)
                    t3 = thinE_t()
                    nc.vector.tensor_tensor(t3[:], Hp(a, 2), Hp(b, 2), OP.mult)
                    At = named(f"A{a}{b}")
                    nc.vector.tensor_tensor(At[:], t1[:], t3[:], OP.add)
                    A6[(a, b)] = At
                a00, a01, a02 = A6[(0, 0)], A6[(0, 1)], A6[(0, 2)]
                a11, a12, a22 = A6[(1, 1)], A6[(1, 2)], A6[(2, 2)]

                # --------- eigenvalues (closed form)
                q3 = thinE_t()
                nc.vector.tensor_tensor(q3[:], a00[:], a11[:], OP.add)
                nc.vector.tensor_tensor(q3[:], q3[:], a22[:], OP.add)
                m01, g0, g1 = named("m01"), named("g0"), named("g1")
                g2 = named("g2")
                nc.vector.tensor_tensor(m01[:], a01[:], a01[:], OP.mult)
                nc.vector.tensor_tensor(g0[:], a01[:], a12[:], OP.mult)
                nc.vector.tensor_tensor(g1[:], a01[:], a02[:], OP.mult)
                nc.vector.tensor_tensor(g2[:], a02[:], a12[:], OP.mult)
                m02 = thinE_t()
                nc.vector.tensor_tensor(m02[:], a02[:], a02[:], OP.mult)
                m12 = thinE_t()
                nc.vector.tensor_tensor(m12[:], a12[:], a12[:], OP.mult)
                p1 = thinE_t()
                nc.vector.tensor_tensor(p1[:], m01[:], m02[:], OP.add)
                nc.vector.tensor_tensor(p1[:], p1[:], m12[:], OP.add)
                q = named("q")
                nc.vector.tensor_scalar_mul(q[:], q3[:], 1.0 / 3)
                b00, b11, b22 = thinE_t(), thinE_t(), thinE_t()
                nc.vector.tensor_tensor(b00[:], a00[:], q[:], OP.subtract)
                nc.vector.tensor_tensor(b11[:], a11[:], q[:], OP.subtract)
                nc.vector.tensor_tensor(b22[:], a22[:], q[:], OP.subtract)
                p2s = thinE_t()
                nc.vector.tensor_tensor(p2s[:], b00[:], b00[:], OP.mult)
                tb = thinE_t()
                nc.vector.tensor_tensor(tb[:], b11[:], b11[:], OP.mult)
                nc.vector.tensor_tensor(p2s[:], p2s[:], tb[:], OP.add)
                nc.vector.tensor_tensor(tb[:], b22[:], b22[:], OP.mult)
                nc.vector.tensor_tensor(p2s[:], p2s[:], tb[:], OP.add)
                nc.vector.scalar_tensor_tensor(
                    p2s[:], p1[:], 2.0, p2s[:], OP.mult, OP.add)
                pA = named("pA")
                nc.scalar.activation(pA[:], p2s[:], AF.Sqrt, scale=1.0 / 6)
                # fill: detB terms (independent of pA)
                c0 = thinE_t()
                nc.vector.tensor_tensor(c0[:], b11[:], b22[:], OP.mult)
                nc.vector.tensor_tensor(c0[:], c0[:], m12[:], OP.subtract)
                c1 = thinE_t()
                nc.vector.tensor_tensor(c1[:], a01[:], b22[:], OP.mult)
                nc.vector.tensor_tensor(c1[:], c1[:], g2[:], OP.subtract)
                c2 = thinE_t()
                nc.vector.tensor_tensor(c2[:], b11[:], a02[:], OP.mult)
                nc.vector.tensor_tensor(c2[:], g0[:], c2[:], OP.subtract)
                detB = thinE_t()
                nc.vector.tensor_tensor(detB[:], b00[:], c0[:], OP.mult)
                tdb = thinE_t()
                nc.vector.tensor_tensor(tdb[:], a01[:], c1[:], OP.mult)
                nc.vector.tensor_tensor(detB[:], detB[:], tdb[:], OP.subtract)
                nc.vector.tensor_tensor(tdb[:], a02[:], c2[:], OP.mult)
                nc.vector.tensor_tensor(detB[:], detB[:], tdb[:], OP.add)
                pinv = thinE_t()
                nc.vector.tensor_scalar_add(pinv[:], pA[:], TINY)
                nc.vector.reciprocal_approx_fast(pinv[:], pinv[:])
                p3 = thinE_t()
                nc.vector.tensor_tensor(p3[:], pinv[:], pinv[:], OP.mult)
                nc.vector.tensor_tensor(p3[:], p3[:], pinv[:], OP.mult)
                rc = thinE_t()
                nc.vector.tensor_tensor(rc[:], detB[:], p3[:], OP.mult)
                nc.vector.tensor_scalar(rc[:], rc[:], 0.5, 1.0, OP.mult, OP.min)
                nc.vector.tensor_scalar_max(rc[:], rc[:], -1.0)
                rr = thinE_t()
                nc.vector.tensor_tensor(rr[:], rc[:], rc[:], OP.mult)
                wA = thinE_t()
                nc.scalar.activation(wA[:], rr[:], AF.Sqrt, bias=1.0, scale=-1.0)
                # fill: pn tree
                pn = psum_t("pn")
                ntree(sp2, pn)
                rat = thinE_t()
                nc.vector.tensor_scalar_add(rat[:], wA[:], 1e-10)
                nc.vector.reciprocal_approx_fast(rat[:], rat[:])
                nc.vector.tensor_tensor(rat[:], rc[:], rat[:], OP.mult)
                # atan with range reduction: |x|>1 -> sign(x)*pi/2 - atan(1/x)
                a1 = thinE_t()
                nc.vector.tensor_scalar(a1[:], rat[:], 1.0, -1.0, OP.min, OP.max)
                rat2 = thinE_t()
                nc.vector.tensor_tensor(rat2[:], rat[:], rat[:], OP.mult)
                rinv = thinE_t()
                nc.vector.tensor_scalar_add(rinv[:], rat2[:], TINY)
                nc.vector.reciprocal_approx_fast(rinv[:], rinv[:])
                nc.vector.tensor_tensor(rinv[:], rat[:], rinv[:], OP.mult)
                nc.vector.tensor_scalar(rinv[:], rinv[:], 1.0, -1.0, OP.min, OP.max)
                sg = thinE_t()
                nc.vector.tensor_scalar(sg[:], rat[:], 1e10, 1.0, OP.mult, OP.min)
                nc.vector.tensor_scalar_max(sg[:], sg[:], -1.0)
                at1 = thinE_t()
                nc.scalar.activation(at1[:], a1[:], AF.Arctan)
                at2 = thinE_t()
                nc.scalar.activation(at2[:], rinv[:], AF.Arctan)
                atb = thinE_t()
                nc.vector.scalar_tensor_tensor(
                    atb[:], sg[:], 1.5707963267948966, at2[:],
                    OP.mult, OP.subtract)
                m_ = thinE_t()
                nc.vector.tensor_scalar_add(m_[:], rat2[:], -1.0)
                nc.vector.tensor_scalar(m_[:], m_[:], 1e10, 1.0, OP.mult, OP.min)
                nc.vector.tensor_scalar_max(m_[:], m_[:], 0.0)
                atn = thinE_t()
                nc.vector.tensor_tensor(atn[:], atb[:], at1[:], OP.subtract)
                nc.vector.tensor_tensor(atn[:], atn[:], m_[:], OP.mult)
                nc.vector.tensor_tensor(atn[:], atn[:], at1[:], OP.add)
                # fill: tn tree
                tn = psum_t("tn")
                ntree(st2, tn)
                cs1 = psum_t("cs1")
                nc.scalar.activation(cs1[:], atn[:], AF.Sin,
                                     bias=b2p3[:], scale=-1.0 / 3)
                cs2 = psum_t("cs2")
                nc.scalar.activation(cs2[:], atn[:], AF.Sin,
                                     bias=b4p3[:], scale=-1.0 / 3)
                # fill: s, s2, P~2 = s^2*P2 into d2 (f32), then d2 += T2
                sS = named("sS")
                nc.vector.tensor_scalar_add(sS[:], pn[:], EPS)
                nc.vector.reciprocal_approx_fast(sS[:], sS[:])
                nc.vector.tensor_tensor(sS[:], sS[:], tn[:], OP.mult)
                s2 = psum_t("s2")
                nc.vector.tensor_tensor(s2[:], sS[:], sS[:], OP.mult)
                nc.vector.tensor_tensor(
                    d2[:], P2[:], _ap(s2, 0, [[0, J], [1, F]]), OP.mult)
                nc.vector.tensor_tensor(d2[:], d2[:], T2[:], OP.add)
                lam0, lam1 = psum_t("lam0"), psum_t("lam1")
                tp = thinE_t()
                nc.vector.tensor_tensor(tp[:], pA[:], cs1[:], OP.mult)
                nc.vector.scalar_tensor_tensor(
                    lam0[:], tp[:], 2.0, q[:], OP.mult, OP.add)
                lam2 = thinE_t()
                nc.vector.tensor_tensor(tp[:], pA[:], cs2[:], OP.mult)
                nc.vector.scalar_tensor_tensor(
                    lam2[:], tp[:], -2.0, q[:], OP.mult, OP.add)
                nc.vector.scalar_tensor_tensor(
                    lam1[:], q[:], 3.0, lam0[:], OP.mult, OP.subtract)
                nc.vector.tensor_tensor(lam1[:], lam1[:], lam2[:], OP.subtract)

                # --------- eigenvectors v0 (lam0), v1 (lam1); v2 = v0 x v1
                def eigvec(lam, pref):
                    vx = named(pref + "x")
                    vy = named(pref + "y")
                    vz = named(pref + "z")
                    b0 = thinE_t()
                    nc.vector.tensor_tensor(b0[:], a00[:], lam[:], OP.subtract)
                    b1 = thinE_t()
                    nc.vector.tensor_tensor(b1[:], a11[:], lam[:], OP.subtract)
                    nc.vector.tensor_tensor(vx[:], a02[:], b1[:], OP.mult)
                    nc.vector.tensor_tensor(vx[:], g0[:], vx[:], OP.subtract)
                    nc.vector.tensor_tensor(vy[:], b0[:], a12[:], OP.mult)
                    nc.vector.tensor_tensor(vy[:], g1[:], vy[:], OP.subtract)
                    nc.vector.tensor_tensor(vz[:], b0[:], b1[:], OP.mult)
                    nc.vector.tensor_tensor(vz[:], vz[:], m01[:], OP.subtract)
                    n2 = thinE_t()
                    nc.vector.tensor_tensor(n2[:], vx[:], vx[:], OP.mult)
                    t2_ = thinE_t()
                    nc.vector.tensor_tensor(t2_[:], vy[:], vy[:], OP.mult)
                    nc.vector.tensor_tensor(n2[:], n2[:], t2_[:], OP.add)
                    nc.vector.tensor_tensor(t2_[:], vz[:], vz[:], OP.mult)
                    nc.vector.tensor_tensor(n2[:], n2[:], t2_[:], OP.add)
                    ns = thinE_t()
                    nc.scalar.activation(ns[:], n2[:], AF.Sqrt)
                    nc.vector.tensor_scalar_add(ns[:], ns[:], TINY)
                    nc.vector.reciprocal_approx_fast(ns[:], ns[:])
                    nc.vector.tensor_tensor(vx[:], vx[:], ns[:], OP.mult)
                    nc.vector.tensor_tensor(vy[:], vy[:], ns[:], OP.mult)
                    nc.vector.tensor_tensor(vz[:], vz[:], ns[:], OP.mult)
                    return vx, vy, vz

                v0 = eigvec(lam0, "v0")
                v1 = eigvec(lam1, "v1")
                v2 = (named("v2x"), named("v2y"), named("v2z"))
                cr = ((1, 2), (2, 0), (0, 1))
                for r in range(3):
                    i1, i2 = cr[r]
                    t1 = thinE_t()
                    nc.vector.tensor_tensor(t1[:], v0[i1][:], v1[i2][:], OP.mult)
                    t2_ = thinE_t()
                    nc.vector.tensor_tensor(t2_[:], v0[i2][:], v1[i1][:], OP.mult)
                    nc.vector.tensor_tensor(v2[r][:], t1[:], t2_[:], OP.subtract)

                # --------- rsig_i = s / sigma_i ; u_i = H v_i * rsig_i
                rsig = []
                for i, lam in enumerate((lam0, lam1)):
                    rl = thinE_t()
                    nc.scalar.activation(rl[:], lam[:], AF.Relu)
                    sg = thinE_t()
                    nc.scalar.activation(sg[:], rl[:], AF.Sqrt)
                    nc.vector.tensor_scalar_add(sg[:], sg[:], TINY)
                    nc.vector.reciprocal_approx_fast(sg[:], sg[:])
                    rs = psum_t(f"rs{i}")
                    nc.vector.scalar_tensor_tensor(
                        rs[:], sg[:], -2.0, sS[:], OP.mult, OP.mult)
                    rsig.append(rs)

                ub = meanp.tile([P, 2304], f32, tag="mt", name="ublock")
                u0 = _ap(ub, 0, [[F, 3], [1, F]])
                u1 = _ap(ub, 3 * F, [[F, 3], [1, F]])
                u2 = _ap(ub, 6 * F, [[F, 3], [1, F]])

                def up(ui, r):
                    return _pl(ub, ui * 3 * F + r * F, F)

                def bc3(t):
                    return _ap(t, 0, [[0, 3], [1, F]])

                def HCg(k):
                    # H rows group for fixed k: planes (c*3+k)... careful:
                    # u_i[r] = sum_k H[r,k] v_i[k]; H[r,k] stored plane (r*3+k)?
                    # H plane (c*3+r) = H_cr = sum_j PC_c TC_r -> H[c,r].
                    # reference H_ik = sum_j pc_{j,i} tc_{j,k} -> H[i,k] = plane(i*3+k)
                    # u_i[r] = sum_k H[r,k] (v_i)_k: planes (r*3+k), r varies
                    # group for fixed k over r: offset k*F stride 3F
                    return _ap(H, k * F, [[3 * F, 3], [1, F]])

                uga = {"v": nc.vector, "g": nc.gpsimd}[kn["uassm"]]
                for i, (vv, rs) in enumerate(((v0, rsig[0]), (v1, rsig[1]))):
                    udst = (u0, u1)[i]
                    gt = meanp.tile([P, 768], f32, tag="mp", name="gt", bufs=1)
                    uga.tensor_tensor(udst, HCg(0), bc3(vv[0]), OP.mult)
                    uga.tensor_tensor(gt[:], HCg(1), bc3(vv[1]), OP.mult)
                    uga.tensor_tensor(udst, udst, gt[:], OP.add)
                    uga.tensor_tensor(gt[:], HCg(2), bc3(vv[2]), OP.mult)
                    uga.tensor_tensor(udst, udst, gt[:], OP.add)
                    uga.tensor_tensor(udst, udst, bc3(rs), OP.mult)
                # u2 = cross(u0, u1) / s
                invs = psum_t("invs")
                nc.vector.tensor_scalar_add(invs[:], sS[:], TINY)
                nc.vector.reciprocal_approx_fast(invs[:], invs[:])
                nc.vector.tensor_scalar_mul(invs[:], invs[:], -0.5)
                for r in range(3):
                    i1, i2 = cr[r]
                    t1 = thinE_t()
                    nc.vector.tensor_tensor(t1[:], up(0, i1), up(1, i2), OP.mult)
                    t2_ = thinE_t()
                    nc.vector.tensor_tensor(t2_[:], up(0, i2), up(1, i1), OP.mult)
                    nc.vector.tensor_tensor(t1[:], t1[:], t2_[:], OP.subtract)
                    nc.vector.tensor_tensor(up(2, r), t1[:], invs[:], OP.mult)

                # --------- G: plane (c*3+r) = sum_i u_i[r] * (v_c)_i, then *-2
                gga = {"v": nc.vector, "g": nc.gpsimd}[kn["gassm"]]
                vs = (v0, v1, v2)
                greps = {}
                for cc in range(3):
                    Gc = _ap(G, cc * 3 * F, [[F, 3], [1, F]])
                    gt = meanp.tile([P, 768], f32, tag="mp", name="gt2", bufs=1)
                    gt2 = meanp.tile([P, 768], f32, tag="mq", name="gt3", bufs=1)
                    gga.tensor_tensor(gt[:], u0, bc3(vs[cc][0]), OP.mult)
                    gga.tensor_tensor(gt2[:], u1, bc3(vs[cc][1]), OP.mult)
                    gga.tensor_tensor(gt[:], gt[:], gt2[:], OP.add)
                    gga.tensor_tensor(gt2[:], u2, bc3(vs[cc][2]), OP.mult)
                    gga.tensor_tensor(Gc, gt[:], gt2[:], OP.add)
                    # replicate this block's planes (r=cc, c=0..2) over j via DMA
                    for c_ in range(3):
                        gr = (oring.tile([P, JF], bf16, tag="O", name="gr",
                                         bufs=1) if c_ == 0 else
                              sqp.tile([P, JF], bf16, tag="sq", name="gr",
                                       bufs=2))
                        nc.sync.dma_start(
                            gr[:], _ap(G, (cc * 3 + c_) * F, [[0, J], [1, F]]))
                        greps[(c_, cc)] = gr

                if stop <= 2:
                    return
                # --------- e_r = sum_c Gt[c*3+r] (bcast over j) * PC_c
                def Gb(cc, r):
                    # G plane (a*3+b) holds (U M)_{b,a}; e_r needs (U M)_{cc,r}
                    if kn.get("edummy"):
                        return cblk(TC, cc)  # timing-only: plain operand
                    return _ap(G, (r * 3 + cc) * F, [[0, J], [1, F]])

                # plain mults from DMA-replicated G planes
                for r in range(3):
                    er = epool.tile([P, JF], bf16, tag="e", name="er", bufs=3)
                    tmp = epool.tile([P, JF], bf16, tag="e", name="etmp", bufs=3)
                    nc.vector.tensor_tensor(
                        er[:], cblk(PC, 0), greps[(0, r)][:], OP.mult)
                    nc.vector.tensor_tensor(
                        tmp[:], cblk(PC, 1), greps[(1, r)][:], OP.mult)
                    nc.vector.tensor_tensor(er[:], er[:], tmp[:], OP.add)
                    nc.vector.tensor_tensor(
                        tmp[:], cblk(PC, 2), greps[(2, r)][:], OP.mult)
                    nc.vector.tensor_tensor(er[:], er[:], tmp[:], OP.add)
                    Ei("tcmul", r).tensor_tensor(
                        er[:], er[:], cblk(TC, r), OP.mult)
                    Ei("d2add", r).tensor_tensor(d2[:], d2[:], er[:], OP.add)

                if stop <= 3:
                    return
                # --------- dist = sqrt(relu(d2)); sum over j; accumulate
                dr = sqp.tile([P, JF], bf16, tag="sq", name="dr", bufs=2)
                nc.scalar.activation(dr[:], d2[:], AF.Relu)
                nc.scalar.activation(dr[:], dr[:], AF.Sqrt)
                dsum = thinE_t()
                ntree(dr, dsum)
                nc.vector.tensor_tensor(acc[:], acc[:], dsum[:], OP.add)

            if iters == 1:
                body()
            else:
                with tc.For_i(0, iters, 1):
                    body()

            accs = persist.tile([P, 1], f32, tag="accs", name="accs")
            nc.vector.tensor_reduce(accs[:], acc[:], axis=AX.X, op=OP.add)
            nc.sync.dma_start(out_d[:], accs[:])

    nc.compile()
    return nc


_nc_cache = None


def get_nc():
    global _nc_cache
    if _nc_cache is None:
        _nc_cache = build_nc()
    return _nc_cache


def run(nc, pred, target, trace=False, **kw):
    pred2 = np.ascontiguousarray(np.asarray(pred), np.float32).reshape(B, JC)
    targ2 = np.ascontiguousarray(np.asarray(target), np.float32).reshape(B, JC)
    in_maps = [
        {"pred": pred2[c * BC:(c + 1) * BC], "target": targ2[c * BC:(c + 1) * BC]}
        for c in range(NCORES)
    ]
    res = run_bass_kernel_spmd(nc, in_maps, list(range(NCORES)), trace=trace, **kw)
    total = sum(r["partial"].astype(np.float64).sum() for r in res.results)
    loss = np.float32(total / (B * J))
    return loss, res


def kernel(pred, target):
    loss, _ = run(get_nc(), pred, target)
    return loss
